# revision 24
# baseline (speedup 1.0000x reference)
"""Trainium2 Bass kernel for nn_DressedQuantumNet.

Math reformulation (exact, up to float rounding):
  pre_out = x @ pre_w.T + pre_b                  # [B,4]
  theta_w = (pi/4)*tanh(pre_out_w) + pi/4        # in (0, pi/2)
  v_w     = [cos theta_w, sin theta_w]           # per-qubit state (positive)
  psi     = v_0 (x) v_1 (x) v_2 (x) v_3          # [B,16] product state
  phi     = M @ psi        # M = fixed 16x16 matrix of the CNOT/RY circuit
  out     = (phi*phi)^T P + post_b  # P[i,c] = sum_w post_w[c,w] * z_w(i)

Device strategy (pure data parallel over 8 cores, 8192 samples each):
  - x is transposed + downcast to fp16 on the HOST, so the contraction dim
    (D=512, 4 chunks of 128) lands on SBUF partitions via plain contiguous
    DMAs (no device-side transpose of the big tensor at all).  The x stream
    is split across the SP HWDGE queue (k=0..2, ~330GB/s) and the gpsimd
    SWDGE queue (k=3, ~135GB/s); the scalar HWDGE ring carries ONLY the
    xbar transposes (a plain-copy->transpose transition drains the ring).
  - all small constants ride in TWO batched DMAs at the head of the SP
    queue (one fp16 image, one fp32 image).
  - pre-matmul: lhsT = tiny pre_w chunk [128d, 4q] (LDWEIGHTS), rhs = xT
    chunk [128d, 512samples] streaming at 1 col/cycle, PSUM accum over k.
  - PSUM evacuation fused with the pre_b bias add on DVE, downcast fp16,
    into rows 0:4 of a [16, B] staging tile (rows 4:16 stay uninitialized;
    engine writes must start at partition 0, and the xbar just moves
    bytes).  One SBUF->SBUF xbar transpose per block ([16,2048]->[128,256],
    on the scalar HWDGE ring -- the only transposes in the kernel).
  - tanh runs AFTER the transpose on all 128 partitions (free size 64),
    then two Sin activations produce cos/sin with folded scale+bias.
  - psi built with 3 broadcast-AP vector multiplies (fp16 out).
  - quantum circuit: PE transpose of psi -> [(tile,comp), sample], then two
    block-diagonal fp16 matmuls (M and P, 8 tiles per 128-wide matmul).
  - output staged transposed in SBUF [80, 1024]; ONE store at the end;
    host undoes the (block, group, half, k) sample permutation.
"""

import os
import sys

for _p in ("/opt/trn_rl_repo",):
    if os.path.isdir(_p) and _p not in sys.path:
        sys.path.insert(0, _p)

import math
import numpy as np
import ml_dtypes
from contextlib import ExitStack

import concourse.bass as bass
import concourse.bacc as bacc
import concourse.mybir as mybir
from concourse.tile import TileContext, add_dep_helper
from concourse.bass_utils import run_bass_kernel_spmd

F32 = mybir.dt.float32
F16 = mybir.dt.float16
AF = mybir.ActivationFunctionType
ALU = mybir.AluOpType
PI4 = math.pi / 4.0

N_CORES = 8
B_FULL, D, C = 65536, 512, 10
B = B_FULL // N_CORES          # 8192 samples per core
N_QUBITS, Q_DEPTH = 4, 6

# x DMA slices along the sample axis (fine early for a fast pipeline start).
# Each (k, slice) gets its OWN SBUF tile: the tile framework tracks deps per
# tile, and a shared tile serializes compute reads against later DMA writes.
SLICES = [(0, 1024), (1024, 2560), (2560, 5120), (5120, 8192)]

# groups per phase-2 block (16 sample tiles each)
BLOCK_GROUPS = [4, 4, 4, 4]


def _slice_of(g):
    s0 = 512 * g
    for si, (c0, c1) in enumerate(SLICES):
        if c0 <= s0 < c1:
            return si, s0 - c0
    raise ValueError(g)


# ---------------------------------------------------------------- host math
def _apply_1q(state, gate, wire):
    state = np.moveaxis(state, wire, 0)
    state = np.tensordot(gate, state, axes=((1,), (0,)))
    return np.moveaxis(state, 0, wire)


def _apply_cnot(state, ctrl, tgt):
    state = np.moveaxis(state, (ctrl, tgt), (0, 1))
    state = np.stack([state[0], state[1][::-1]], axis=0)
    return np.moveaxis(state, (0, 1), (ctrl, tgt))


def _ry(theta):
    c, s = np.cos(theta * 0.5), np.sin(theta * 0.5)
    return np.array([[c, -s], [s, c]])


def _build_M(q_params: np.ndarray) -> np.ndarray:
    """16x16 matrix of the fixed part of the circuit (after the per-sample
    RY layer): 6 repetitions of [CNOT(0,1), CNOT(2,3), CNOT(1,2), RY layer]."""
    qw = np.asarray(q_params, np.float64).reshape(Q_DEPTH, N_QUBITS)
    M = np.zeros((16, 16), np.float64)
    for i in range(16):
        state = np.zeros(16, np.float64)
        state[i] = 1.0
        state = state.reshape((2,) * N_QUBITS)
        for k in range(Q_DEPTH):
            for a in range(0, N_QUBITS - 1, 2):
                state = _apply_cnot(state, a, a + 1)
            for a in range(1, N_QUBITS - 1, 2):
                state = _apply_cnot(state, a, a + 1)
            for w in range(N_QUBITS):
                state = _apply_1q(state, _ry(qw[k, w]), w)
        M[:, i] = state.reshape(16)
    return M


def _build_P(post_w: np.ndarray) -> np.ndarray:
    """P[i, c] = sum_w post_w[c, w] * z_w(i), where z_w(i) flips sign with
    bit (3-w) of the state index i (axis 0 of the state = qubit 0)."""
    post_w = np.asarray(post_w, np.float64)
    i = np.arange(16)
    z = np.stack([1.0 - 2.0 * ((i >> (3 - w)) & 1) for w in range(N_QUBITS)], 1)
    return z @ post_w.T  # [16, 10]


# ---------------------------------------------------------------- bass build
def build_nc() -> bass.Bass:
    # Bacc (not raw Bass): its finalize() runs generate_event_semaphores,
    # which splits multi-semaphore waits to satisfy the TRN2 one-wait-per-
    # instruction ISA limit.
    nc = bacc.Bacc(None)
    x4 = nc.dram_tensor("x4", [4, 128, B], F16, kind="ExternalInput")
    cst16 = nc.dram_tensor("cst16", [128, 352], F16, kind="ExternalInput")
    cst32 = nc.dram_tensor("cst32", [128, 132], F32, kind="ExternalInput")
    # transposed layout; host flips back (see unpack_out)
    out = nc.dram_tensor("out", [80, 1024], F32, kind="ExternalOutput")

    with ExitStack() as ctx:
        tc = ctx.enter_context(TileContext(nc))
        consts = ctx.enter_context(tc.tile_pool(name="consts", bufs=1))
        work = ctx.enter_context(tc.tile_pool(name="work", bufs=3))
        ps_po = ctx.enter_context(tc.tile_pool(name="ps_po", space="PSUM", bufs=3))
        ps2 = ctx.enter_context(tc.tile_pool(name="ps2", space="PSUM", bufs=4))

        # --- persistent SBUF ---
        cst16_sb = consts.tile([128, 352], F16)
        cst32_sb = consts.tile([128, 132], F32)
        xts = [[consts.tile([128, c1 - c0], F16, name=f"xt{k}_{si}")
                for si, (c0, c1) in enumerate(SLICES)] for k in range(4)]
        # rows 0:4 = qubits (rows 4:16 never written: engine writes must
        # start at partition 0; the xbar transpose just moves their bytes)
        t16 = consts.tile([16, B], F16)
        out2 = consts.tile([80, 128 * (B // 1024)], F32)
        warm = consts.tile([1, 2], F32)

        pre_wt = cst16_sb[:, 0:16]     # [p, 4k+q] = pre_w[q, 128k+p]
        mbd = cst16_sb[:, 16:144]      # block-diag 8 x M^T
        pbd = cst16_sb[:, 144:224]     # block-diag 8 x P
        pre_b = cst32_sb[0:4, 0:1]
        pb80 = cst32_sb[0:80, 1:2]
        bias_cos = cst32_sb[:, 2:3]    # 3*pi/4
        bias_sin = cst32_sb[:, 3:4]    # pi/4
        ident32 = cst32_sb[:, 4:132]

        # --- consts ride the gpsimd SWDGE path (own semaphore pool, off
        # the 8 shared HWDGE lanes)
        nc.gpsimd.dma_start(cst16_sb, cst16[:, :])
        nc.gpsimd.dma_start(cst32_sb, cst32[:, :])

        # --- the x stream: k0,k1 on the SP HWDGE ring; k2,k3 on the
        # gpsimd SWDGE queue (its own sem pool and ring).  The ACT HWDGE
        # ring carries ONLY the xbar transposes: a plain-copy -> transpose
        # transition drains the whole ring, so ANY plain copy there would
        # serialize every phase-2 chain behind the full x stream.
        for si, (c0, c1) in enumerate(SLICES):
            nc.scalar.dma_start(xts[3][si], x4[3, :, c0:c1])
            nc.sync.dma_start(xts[0][si], x4[0, :, c0:c1])
            nc.sync.dma_start(xts[1][si], x4[1, :, c0:c1])
            nc.sync.dma_start(xts[2][si], x4[2, :, c0:c1])
            if si == 0:
                # pin the activation table to silu_and_others (the only
                # table with silu; it also has tanh+sin+square+identity, so
                # no further loads).  Reads `warm` itself -- garbage in,
                # garbage out, no DMA dep.
                nc.scalar.activation(warm[:, 0:1], warm[:, 1:2], AF.Silu)

        gbase = [sum(BLOCK_GROUPS[:b]) for b in range(len(BLOCK_GROUPS) + 1)]

        def phase1(blk):
            # pre-net for this block's groups of 512 samples
            for g in range(gbase[blk], gbase[blk + 1]):
                po = ps_po.tile([4, 512], F32, name="po", tag="po")
                si, o = _slice_of(g)
                for k in range(4):
                    nc.tensor.matmul(
                        po[:, :],
                        lhsT=pre_wt[:, 4 * k:4 * (k + 1)],
                        rhs=xts[k][si][:, o:o + 512],
                        start=(k == 0), stop=(k == 3))
                # PSUM evacuation + pre_b bias, fp16
                nc.vector.tensor_scalar(
                    out=t16[0:4, 512 * g:512 * (g + 1)],
                    in0=po[:, :], scalar1=pre_b, scalar2=None, op0=ALU.add)

        def phase2_front(blk):
            # trig + psi for this block; one xbar transpose:
            # [16, 512*ngroups] -> [128, (t, nt)(i, 16)]
            g0, g1 = gbase[blk], gbase[blk + 1]
            nt = 4 * (g1 - g0)              # sample tiles in this block
            th = work.tile([128, 16 * nt], F16, name="th", tag="th")
            nc.scalar.dma_start(
                th[:, :].rearrange("p (t i) -> p t i", i=16),
                t16[:, 512 * g0:512 * g1],
                transpose=True)
            th3 = th[:, :].rearrange("p (t i) -> p t i", i=16)[:, :, 0:4]
            tnh = work.tile([128, 4 * nt], F16, name="tnh", tag="tnh")
            nc.scalar.activation(
                tnh[:, :].rearrange("p (t q) -> p t q", q=4), th3, AF.Tanh)
            # cos(theta) = sin(pi/4*tanh + 3pi/4); sin(theta) = sin(.. + pi/4)
            cs = work.tile([128, 8 * nt], F32, name="cs", tag="cs")
            cs2 = cs[:, :].rearrange("p (t c e x) -> p t c e x", c=2, e=2, x=2)
            tnh3 = tnh[:, :].rearrange("p (t c e) -> p t c e", c=2, e=2)
            nc.scalar.activation(cs2[:, :, :, :, 0], tnh3, AF.Sin,
                                 bias=bias_cos, scale=PI4)
            nc.scalar.activation(cs2[:, :, :, :, 1], tnh3, AF.Sin,
                                 bias=bias_sin, scale=PI4)
            # v[t, c, a, b] = cs[t, qubit 2c, a] * cs[t, qubit 2c+1, b]
            # (c=0 -> qubits 01, c=1 -> qubits 23) in ONE broadcast multiply
            v = work.tile([128, 8 * nt], F32, name="v", tag="v")
            nc.vector.tensor_tensor(
                out=v[:, :].rearrange("p (t c a b) -> p t c a b", c=2, a=2, b=2),
                in0=cs2[:, :, :, 0, :].unsqueeze(4)
                    .broadcast_to((128, nt, 2, 2, 2)),
                in1=cs2[:, :, :, 1, :].unsqueeze(3)
                    .broadcast_to((128, nt, 2, 2, 2)),
                op=ALU.mult)
            v4 = v[:, :].rearrange("p (t c i) -> p t c i", c=2, i=4)
            psi = work.tile([128, 16 * nt], F32, name="psi", tag="psi")
            nc.vector.tensor_tensor(
                out=psi[:, :].rearrange("p (t a b) -> p t a b", a=4, b=4),
                in0=v4[:, :, 0, :].unsqueeze(3).broadcast_to((128, nt, 4, 4)),
                in1=v4[:, :, 1, :].unsqueeze(2).broadcast_to((128, nt, 4, 4)),
                op=ALU.mult)
            return psi

        def phase2_back(blk, psi):
            nh = (gbase[blk + 1] - gbase[blk]) // 2   # halves of 8 tiles
            for h in range(nh):
                hh = gbase[blk] // 2 + h
                psiT_ps = ps2.tile([128, 128], F32, name="psiT_ps", tag="p2")
                nc.tensor.transpose(
                    psiT_ps, psi[:, 128 * h:128 * (h + 1)], ident32)
                psiT = work.tile([128, 128], F16, name="psiT", tag="psiT")
                nc.vector.tensor_copy(psiT, psiT_ps)
                phi_ps = ps2.tile([128, 128], F32, name="phi_ps", tag="p2")
                nc.tensor.matmul(phi_ps, lhsT=mbd, rhs=psiT,
                                 start=True, stop=True)
                phi2 = work.tile([128, 128], F16, name="phi2", tag="phi2")
                nc.scalar.activation(phi2, phi_ps, AF.Square)
                o10 = ps2.tile([80, 128], F32, name="o10", tag="p2")
                nc.tensor.matmul(o10, lhsT=pbd, rhs=phi2,
                                 start=True, stop=True)
                nc.vector.tensor_scalar(
                    out=out2[:, 128 * hh:128 * (hh + 1)],
                    in0=o10[:, :], scalar1=pb80, scalar2=None, op0=ALU.add)

        def store(blk):
            h0, h1 = gbase[blk] // 2, gbase[blk + 1] // 2
            nc.sync.dma_start(out[:, 128 * h0:128 * h1],
                              out2[:, 128 * h0:128 * h1])

        NB = len(BLOCK_GROUPS)
        psis = {}
        phase1(0)
        psis[0] = phase2_front(0)
        for blk in range(1, NB):
            phase1(blk)
            phase2_back(blk - 1, psis.pop(blk - 1))
            store(blk - 1)
            psis[blk] = phase2_front(blk)
        phase2_back(NB - 1, psis.pop(NB - 1))
        store(NB - 1)

    nc.finalize()  # bacc: register alloc + event-semaphore wait splitting
    return nc


_NC_CACHE: dict = {}


def _get_nc() -> bass.Bass:
    if "nc" not in _NC_CACHE:
        _NC_CACHE["nc"] = build_nc()
    return _NC_CACHE["nc"]


def make_in_maps(inputs: dict) -> list:
    x = np.asarray(inputs["input_features"], np.float32)
    pre_w = np.asarray(inputs["pre_w"], np.float32)
    pre_b = np.asarray(inputs["pre_b"], np.float32)
    q_params = np.asarray(inputs["q_params"], np.float32)
    post_w = np.asarray(inputs["post_w"], np.float32)
    post_b = np.asarray(inputs["post_b"], np.float32)

    M = _build_M(q_params)
    P = _build_P(post_w)
    mbd = np.zeros((128, 128), np.float64)
    pbd = np.zeros((128, 80), np.float64)
    for t in range(8):
        mbd[16 * t:16 * (t + 1), 16 * t:16 * (t + 1)] = M.T
        pbd[16 * t:16 * (t + 1), 10 * t:10 * (t + 1)] = P
    # pre_wt[p, 4k+q] = pre_w[q, 128k+p]
    pre_wt = np.ascontiguousarray(
        pre_w.T.reshape(4, 128, 4).transpose(1, 0, 2).reshape(128, 16))
    cst16 = np.zeros((128, 352), np.float32)
    cst16[:, 0:16] = pre_wt
    cst16[:, 16:144] = mbd
    cst16[:, 144:224] = pbd
    cst16[:, 224:352] = np.eye(128)
    cst16 = cst16.astype(np.float16)

    cst32 = np.zeros((128, 132), np.float32)
    cst32[0:4, 0] = pre_b
    cst32[0:80, 1] = np.tile(post_b, 8)
    cst32[:, 2] = 3.0 * PI4
    cst32[:, 3] = PI4
    cst32[:, 4:132] = np.eye(128)
    cst32 = np.ascontiguousarray(cst32)

    xh = x.astype(np.float16)
    maps = []
    for i in range(N_CORES):
        xc = np.ascontiguousarray(xh[B * i:B * (i + 1)].T).reshape(4, 128, B)
        maps.append(dict(x4=xc, cst16=cst16, cst32=cst32))
    return maps


def unpack_out(dev: np.ndarray) -> np.ndarray:
    """[80, 1024] device layout -> [B, C].

    dev[10*t8 + c, 128*hh + s] is the output for class c of
    sample 128*(8*hh + t8) + s.
    """
    a = dev.reshape(8, 10, 8, 128)              # (t8, c, hh, s)
    return np.ascontiguousarray(
        a.transpose(2, 0, 3, 1).reshape(B, C))


def run_on_device(inputs: dict, **kwargs):
    """Returns (full_output, BassKernelResults)."""
    nc = _get_nc()
    in_maps = make_in_maps(inputs)
    res = run_bass_kernel_spmd(nc, in_maps, core_ids=list(range(N_CORES)),
                               **kwargs)
    full = np.concatenate(
        [unpack_out(res.results[i]["out"]) for i in range(N_CORES)], 0)
    return np.ascontiguousarray(full, dtype=np.float32), res


def kernel(**inputs) -> np.ndarray:
    out, _ = run_on_device(inputs)
    return out


# revision 27
# speedup vs baseline: 1.0427x; 1.0427x over previous
"""Trainium2 Bass kernel for nn_DressedQuantumNet.

Math reformulation (exact, up to float rounding):
  pre_out = x @ pre_w.T + pre_b                  # [B,4]
  theta_w = (pi/4)*tanh(pre_out_w) + pi/4        # in (0, pi/2)
  v_w     = [cos theta_w, sin theta_w]           # per-qubit state (positive)
  psi     = v_0 (x) v_1 (x) v_2 (x) v_3          # [B,16] product state
  phi     = M @ psi        # M = fixed 16x16 matrix of the CNOT/RY circuit
  out     = (phi*phi)^T P + post_b  # P[i,c] = sum_w post_w[c,w] * z_w(i)

Device strategy (pure data parallel over 8 cores, 8192 samples each):
  - x is transposed + downcast to fp16 on the HOST, so the contraction dim
    (D=512, 4 chunks of 128) lands on SBUF partitions via plain contiguous
    DMAs (no device-side transpose of the big tensor at all).  The x stream
    is split across the SP HWDGE queue (k=0..2, ~330GB/s) and the gpsimd
    SWDGE queue (k=3, ~135GB/s); the scalar HWDGE ring carries ONLY the
    xbar transposes (a plain-copy->transpose transition drains the ring).
  - all small constants ride in TWO batched DMAs at the head of the SP
    queue (one fp16 image, one fp32 image).
  - pre-matmul: lhsT = tiny pre_w chunk [128d, 4q] (LDWEIGHTS), rhs = xT
    chunk [128d, 512samples] streaming at 1 col/cycle, PSUM accum over k.
  - PSUM evacuation fused with the pre_b bias add on DVE, downcast fp16,
    into rows 0:4 of a [16, B] staging tile (rows 4:16 stay uninitialized;
    engine writes must start at partition 0, and the xbar just moves
    bytes).  One SBUF->SBUF xbar transpose per block ([16,2048]->[128,256],
    on the scalar HWDGE ring -- the only transposes in the kernel).
  - tanh runs AFTER the transpose on all 128 partitions (free size 64),
    then two Sin activations produce cos/sin with folded scale+bias.
  - psi built with 3 broadcast-AP vector multiplies (fp16 out).
  - quantum circuit: PE transpose of psi -> [(tile,comp), sample], then two
    block-diagonal fp16 matmuls (M and P, 8 tiles per 128-wide matmul).
  - output staged transposed in SBUF [80, 1024]; ONE store at the end;
    host undoes the (block, group, half, k) sample permutation.
"""

import os
import sys

for _p in ("/opt/trn_rl_repo",):
    if os.path.isdir(_p) and _p not in sys.path:
        sys.path.insert(0, _p)

import math
import numpy as np
import ml_dtypes
from contextlib import ExitStack

import concourse.bass as bass
import concourse.bacc as bacc
import concourse.mybir as mybir
from concourse.tile import TileContext, add_dep_helper
from concourse.bass_utils import run_bass_kernel_spmd

F32 = mybir.dt.float32
F16 = mybir.dt.float16
AF = mybir.ActivationFunctionType
ALU = mybir.AluOpType
PI4 = math.pi / 4.0

N_CORES = 8
B_FULL, D, C = 65536, 512, 10
B = B_FULL // N_CORES          # 8192 samples per core
N_QUBITS, Q_DEPTH = 4, 6

# x DMA slices along the sample axis (fine early for a fast pipeline start).
# Each (k, slice) gets its OWN SBUF tile: the tile framework tracks deps per
# tile, and a shared tile serializes compute reads against later DMA writes.
SLICES = [(0, 1024), (1024, 2560), (2560, 5120), (5120, 8192)]

# groups per phase-2 block.  The xbar transposes can only run after the
# ACT ring drains its k2/k3 plain copies (copy->transpose transition), so
# the chains start late regardless -- two WIDE blocks amortize the fixed
# per-chain latency (transpose issue + semaphore hops) and pipeline their
# four 8-tile halves across PE/DVE/ACT.
BLOCK_GROUPS = [8, 8]


def _slice_of(g):
    s0 = 512 * g
    for si, (c0, c1) in enumerate(SLICES):
        if c0 <= s0 < c1:
            return si, s0 - c0
    raise ValueError(g)


# ---------------------------------------------------------------- host math
def _apply_1q(state, gate, wire):
    state = np.moveaxis(state, wire, 0)
    state = np.tensordot(gate, state, axes=((1,), (0,)))
    return np.moveaxis(state, 0, wire)


def _apply_cnot(state, ctrl, tgt):
    state = np.moveaxis(state, (ctrl, tgt), (0, 1))
    state = np.stack([state[0], state[1][::-1]], axis=0)
    return np.moveaxis(state, (0, 1), (ctrl, tgt))


def _ry(theta):
    c, s = np.cos(theta * 0.5), np.sin(theta * 0.5)
    return np.array([[c, -s], [s, c]])


def _build_M(q_params: np.ndarray) -> np.ndarray:
    """16x16 matrix of the fixed part of the circuit (after the per-sample
    RY layer): 6 repetitions of [CNOT(0,1), CNOT(2,3), CNOT(1,2), RY layer]."""
    qw = np.asarray(q_params, np.float64).reshape(Q_DEPTH, N_QUBITS)
    M = np.zeros((16, 16), np.float64)
    for i in range(16):
        state = np.zeros(16, np.float64)
        state[i] = 1.0
        state = state.reshape((2,) * N_QUBITS)
        for k in range(Q_DEPTH):
            for a in range(0, N_QUBITS - 1, 2):
                state = _apply_cnot(state, a, a + 1)
            for a in range(1, N_QUBITS - 1, 2):
                state = _apply_cnot(state, a, a + 1)
            for w in range(N_QUBITS):
                state = _apply_1q(state, _ry(qw[k, w]), w)
        M[:, i] = state.reshape(16)
    return M


def _build_P(post_w: np.ndarray) -> np.ndarray:
    """P[i, c] = sum_w post_w[c, w] * z_w(i), where z_w(i) flips sign with
    bit (3-w) of the state index i (axis 0 of the state = qubit 0)."""
    post_w = np.asarray(post_w, np.float64)
    i = np.arange(16)
    z = np.stack([1.0 - 2.0 * ((i >> (3 - w)) & 1) for w in range(N_QUBITS)], 1)
    return z @ post_w.T  # [16, 10]


# ---------------------------------------------------------------- bass build
def build_nc() -> bass.Bass:
    # Bacc (not raw Bass): its finalize() runs generate_event_semaphores,
    # which splits multi-semaphore waits to satisfy the TRN2 one-wait-per-
    # instruction ISA limit.
    nc = bacc.Bacc(None)
    x4 = nc.dram_tensor("x4", [4, 128, B], F16, kind="ExternalInput")
    cst16 = nc.dram_tensor("cst16", [128, 352], F16, kind="ExternalInput")
    cst32 = nc.dram_tensor("cst32", [128, 132], F32, kind="ExternalInput")
    # transposed layout; host flips back (see unpack_out)
    out = nc.dram_tensor("out", [80, 1024], F32, kind="ExternalOutput")

    with ExitStack() as ctx:
        tc = ctx.enter_context(TileContext(nc))
        consts = ctx.enter_context(tc.tile_pool(name="consts", bufs=1))
        work = ctx.enter_context(tc.tile_pool(name="work", bufs=3))
        ps_po = ctx.enter_context(tc.tile_pool(name="ps_po", space="PSUM", bufs=3))
        ps2 = ctx.enter_context(tc.tile_pool(name="ps2", space="PSUM", bufs=4))

        # --- persistent SBUF ---
        cst16_sb = consts.tile([128, 352], F16)
        cst32_sb = consts.tile([128, 132], F32)
        xts = [[consts.tile([128, c1 - c0], F16, name=f"xt{k}_{si}")
                for si, (c0, c1) in enumerate(SLICES)] for k in range(4)]
        # rows 0:4 = qubits (rows 4:16 never written: engine writes must
        # start at partition 0; the xbar transpose just moves their bytes)
        t16 = consts.tile([16, B], F16)
        out2 = consts.tile([80, 128 * (B // 1024)], F32)
        warm = consts.tile([1, 2], F32)

        pre_wt = cst16_sb[:, 0:16]     # [p, 4k+q] = pre_w[q, 128k+p]
        mbd = cst16_sb[:, 16:144]      # block-diag 8 x M^T
        pbd = cst16_sb[:, 144:224]     # block-diag 8 x P
        pre_b = cst32_sb[0:4, 0:1]
        pb80 = cst32_sb[0:80, 1:2]
        bias_cos = cst32_sb[:, 2:3]    # 3*pi/4
        bias_sin = cst32_sb[:, 3:4]    # pi/4
        ident32 = cst32_sb[:, 4:132]

        # --- consts ride the gpsimd SWDGE path (own semaphore pool, off
        # the 8 shared HWDGE lanes)
        nc.gpsimd.dma_start(cst16_sb, cst16[:, :])
        nc.gpsimd.dma_start(cst32_sb, cst32[:, :])

        # --- the x stream: k0,k1 on the SP HWDGE ring; k2,k3 on the
        # gpsimd SWDGE queue (its own sem pool and ring).  The ACT HWDGE
        # ring carries ONLY the xbar transposes: a plain-copy -> transpose
        # transition drains the whole ring, so ANY plain copy there would
        # serialize every phase-2 chain behind the full x stream.
        for si, (c0, c1) in enumerate(SLICES):
            nc.scalar.dma_start(xts[2][si], x4[2, :, c0:c1])
            nc.scalar.dma_start(xts[3][si], x4[3, :, c0:c1])
            nc.sync.dma_start(xts[0][si], x4[0, :, c0:c1])
            nc.sync.dma_start(xts[1][si], x4[1, :, c0:c1])
            if si == 0:
                # pin the activation table to silu_and_others (the only
                # table with silu; it also has tanh+sin+square+identity, so
                # no further loads).  Reads `warm` itself -- garbage in,
                # garbage out, no DMA dep.
                nc.scalar.activation(warm[:, 0:1], warm[:, 1:2], AF.Silu)

        gbase = [sum(BLOCK_GROUPS[:b]) for b in range(len(BLOCK_GROUPS) + 1)]

        def phase1(blk):
            # pre-net for this block's groups of 512 samples
            for g in range(gbase[blk], gbase[blk + 1]):
                po = ps_po.tile([4, 512], F32, name="po", tag="po")
                si, o = _slice_of(g)
                for k in range(4):
                    nc.tensor.matmul(
                        po[:, :],
                        lhsT=pre_wt[:, 4 * k:4 * (k + 1)],
                        rhs=xts[k][si][:, o:o + 512],
                        start=(k == 0), stop=(k == 3))
                # PSUM evacuation + pre_b bias, fp16
                nc.vector.tensor_scalar(
                    out=t16[0:4, 512 * g:512 * (g + 1)],
                    in0=po[:, :], scalar1=pre_b, scalar2=None, op0=ALU.add)

        def phase2_front(blk):
            # trig + psi for this block; one xbar transpose:
            # [16, 512*ngroups] -> [128, (t, nt)(i, 16)]
            g0, g1 = gbase[blk], gbase[blk + 1]
            nt = 4 * (g1 - g0)              # sample tiles in this block
            th = work.tile([128, 16 * nt], F16, name="th", tag="th")
            nc.scalar.dma_start(
                th[:, :].rearrange("p (t i) -> p t i", i=16),
                t16[:, 512 * g0:512 * g1],
                transpose=True)
            th3 = th[:, :].rearrange("p (t i) -> p t i", i=16)[:, :, 0:4]
            tnh = work.tile([128, 4 * nt], F16, name="tnh", tag="tnh")
            nc.scalar.activation(
                tnh[:, :].rearrange("p (t q) -> p t q", q=4), th3, AF.Tanh)
            # cos(theta) = sin(pi/4*tanh + 3pi/4); sin(theta) = sin(.. + pi/4)
            cs = work.tile([128, 8 * nt], F32, name="cs", tag="cs")
            cs2 = cs[:, :].rearrange("p (t c e x) -> p t c e x", c=2, e=2, x=2)
            tnh3 = tnh[:, :].rearrange("p (t c e) -> p t c e", c=2, e=2)
            nc.scalar.activation(cs2[:, :, :, :, 0], tnh3, AF.Sin,
                                 bias=bias_cos, scale=PI4)
            nc.scalar.activation(cs2[:, :, :, :, 1], tnh3, AF.Sin,
                                 bias=bias_sin, scale=PI4)
            # v[t, c, a, b] = cs[t, qubit 2c, a] * cs[t, qubit 2c+1, b]
            # (c=0 -> qubits 01, c=1 -> qubits 23) in ONE broadcast multiply
            v = work.tile([128, 8 * nt], F32, name="v", tag="v")
            nc.vector.tensor_tensor(
                out=v[:, :].rearrange("p (t c a b) -> p t c a b", c=2, a=2, b=2),
                in0=cs2[:, :, :, 0, :].unsqueeze(4)
                    .broadcast_to((128, nt, 2, 2, 2)),
                in1=cs2[:, :, :, 1, :].unsqueeze(3)
                    .broadcast_to((128, nt, 2, 2, 2)),
                op=ALU.mult)
            v4 = v[:, :].rearrange("p (t c i) -> p t c i", c=2, i=4)
            psi = work.tile([128, 16 * nt], F32, name="psi", tag="psi")
            nc.vector.tensor_tensor(
                out=psi[:, :].rearrange("p (t a b) -> p t a b", a=4, b=4),
                in0=v4[:, :, 0, :].unsqueeze(3).broadcast_to((128, nt, 4, 4)),
                in1=v4[:, :, 1, :].unsqueeze(2).broadcast_to((128, nt, 4, 4)),
                op=ALU.mult)
            return psi

        def phase2_back(blk, psi):
            nh = (gbase[blk + 1] - gbase[blk]) // 2   # halves of 8 tiles
            for h in range(nh):
                hh = gbase[blk] // 2 + h
                psiT_ps = ps2.tile([128, 128], F32, name="psiT_ps", tag="p2")
                nc.tensor.transpose(
                    psiT_ps, psi[:, 128 * h:128 * (h + 1)], ident32)
                psiT = work.tile([128, 128], F16, name="psiT", tag="psiT")
                nc.vector.tensor_copy(psiT, psiT_ps)
                phi_ps = ps2.tile([128, 128], F32, name="phi_ps", tag="p2")
                nc.tensor.matmul(phi_ps, lhsT=mbd, rhs=psiT,
                                 start=True, stop=True)
                phi2 = work.tile([128, 128], F16, name="phi2", tag="phi2")
                nc.scalar.activation(phi2, phi_ps, AF.Square)
                o10 = ps2.tile([80, 128], F32, name="o10", tag="p2")
                nc.tensor.matmul(o10, lhsT=pbd, rhs=phi2,
                                 start=True, stop=True)
                nc.vector.tensor_scalar(
                    out=out2[:, 128 * hh:128 * (hh + 1)],
                    in0=o10[:, :], scalar1=pb80, scalar2=None, op0=ALU.add)

        def store(blk):
            h0, h1 = gbase[blk] // 2, gbase[blk + 1] // 2
            nc.sync.dma_start(out[:, 128 * h0:128 * h1],
                              out2[:, 128 * h0:128 * h1])

        NB = len(BLOCK_GROUPS)
        psis = {}
        phase1(0)
        psis[0] = phase2_front(0)
        for blk in range(1, NB):
            phase1(blk)
            phase2_back(blk - 1, psis.pop(blk - 1))
            store(blk - 1)
            psis[blk] = phase2_front(blk)
        phase2_back(NB - 1, psis.pop(NB - 1))
        store(NB - 1)

    nc.finalize()  # bacc: register alloc + event-semaphore wait splitting
    return nc


_NC_CACHE: dict = {}


def _get_nc() -> bass.Bass:
    if "nc" not in _NC_CACHE:
        _NC_CACHE["nc"] = build_nc()
    return _NC_CACHE["nc"]


def make_in_maps(inputs: dict) -> list:
    x = np.asarray(inputs["input_features"], np.float32)
    pre_w = np.asarray(inputs["pre_w"], np.float32)
    pre_b = np.asarray(inputs["pre_b"], np.float32)
    q_params = np.asarray(inputs["q_params"], np.float32)
    post_w = np.asarray(inputs["post_w"], np.float32)
    post_b = np.asarray(inputs["post_b"], np.float32)

    M = _build_M(q_params)
    P = _build_P(post_w)
    mbd = np.zeros((128, 128), np.float64)
    pbd = np.zeros((128, 80), np.float64)
    for t in range(8):
        mbd[16 * t:16 * (t + 1), 16 * t:16 * (t + 1)] = M.T
        pbd[16 * t:16 * (t + 1), 10 * t:10 * (t + 1)] = P
    # pre_wt[p, 4k+q] = pre_w[q, 128k+p]
    pre_wt = np.ascontiguousarray(
        pre_w.T.reshape(4, 128, 4).transpose(1, 0, 2).reshape(128, 16))
    cst16 = np.zeros((128, 352), np.float32)
    cst16[:, 0:16] = pre_wt
    cst16[:, 16:144] = mbd
    cst16[:, 144:224] = pbd
    cst16[:, 224:352] = np.eye(128)
    cst16 = cst16.astype(np.float16)

    cst32 = np.zeros((128, 132), np.float32)
    cst32[0:4, 0] = pre_b
    cst32[0:80, 1] = np.tile(post_b, 8)
    cst32[:, 2] = 3.0 * PI4
    cst32[:, 3] = PI4
    cst32[:, 4:132] = np.eye(128)
    cst32 = np.ascontiguousarray(cst32)

    xh = x.astype(np.float16)
    maps = []
    for i in range(N_CORES):
        xc = np.ascontiguousarray(xh[B * i:B * (i + 1)].T).reshape(4, 128, B)
        maps.append(dict(x4=xc, cst16=cst16, cst32=cst32))
    return maps


def unpack_out(dev: np.ndarray) -> np.ndarray:
    """[80, 1024] device layout -> [B, C].

    dev[10*t8 + c, 128*hh + s] is the output for class c of
    sample 128*(8*hh + t8) + s.
    """
    a = dev.reshape(8, 10, 8, 128)              # (t8, c, hh, s)
    return np.ascontiguousarray(
        a.transpose(2, 0, 3, 1).reshape(B, C))


def run_on_device(inputs: dict, **kwargs):
    """Returns (full_output, BassKernelResults)."""
    nc = _get_nc()
    in_maps = make_in_maps(inputs)
    res = run_bass_kernel_spmd(nc, in_maps, core_ids=list(range(N_CORES)),
                               **kwargs)
    full = np.concatenate(
        [unpack_out(res.results[i]["out"]) for i in range(N_CORES)], 0)
    return np.ascontiguousarray(full, dtype=np.float32), res


def kernel(**inputs) -> np.ndarray:
    out, _ = run_on_device(inputs)
    return out


# revision 28
# speedup vs baseline: 1.1242x; 1.0781x over previous
"""Trainium2 Bass kernel for nn_DressedQuantumNet.

Math reformulation (exact, up to float rounding):
  pre_out = x @ pre_w.T + pre_b                  # [B,4]
  theta_w = (pi/4)*tanh(pre_out_w) + pi/4        # in (0, pi/2)
  v_w     = [cos theta_w, sin theta_w]           # per-qubit state (positive)
  psi     = v_0 (x) v_1 (x) v_2 (x) v_3          # [B,16] product state
  phi     = M @ psi        # M = fixed 16x16 matrix of the CNOT/RY circuit
  out     = (phi*phi)^T P + post_b  # P[i,c] = sum_w post_w[c,w] * z_w(i)

Device strategy (pure data parallel over 8 cores, 8192 samples each):
  - x is transposed + downcast to fp16 on the HOST, so the contraction dim
    (D=512, 4 chunks of 128) lands on SBUF partitions via plain contiguous
    DMAs (no device-side transpose of the big tensor at all).  The x stream
    is split across the SP HWDGE queue (k=0..2, ~330GB/s) and the gpsimd
    SWDGE queue (k=3, ~135GB/s); the scalar HWDGE ring carries ONLY the
    xbar transposes (a plain-copy->transpose transition drains the ring).
  - all small constants ride in TWO batched DMAs at the head of the SP
    queue (one fp16 image, one fp32 image).
  - pre-matmul: lhsT = tiny pre_w chunk [128d, 4q] (LDWEIGHTS), rhs = xT
    chunk [128d, 512samples] streaming at 1 col/cycle, PSUM accum over k.
  - PSUM evacuation fused with the pre_b bias add on DVE, downcast fp16,
    into rows 0:4 of a [16, B] staging tile (rows 4:16 stay uninitialized;
    engine writes must start at partition 0, and the xbar just moves
    bytes).  One SBUF->SBUF xbar transpose per block ([16,2048]->[128,256],
    on the scalar HWDGE ring -- the only transposes in the kernel).
  - tanh runs AFTER the transpose on all 128 partitions (free size 64),
    then two Sin activations produce cos/sin with folded scale+bias.
  - psi built with 3 broadcast-AP vector multiplies (fp16 out).
  - quantum circuit: PE transpose of psi -> [(tile,comp), sample], then two
    block-diagonal fp16 matmuls (M and P, 8 tiles per 128-wide matmul).
  - output staged transposed in SBUF [80, 1024]; ONE store at the end;
    host undoes the (block, group, half, k) sample permutation.
"""

import os
import sys

for _p in ("/opt/trn_rl_repo",):
    if os.path.isdir(_p) and _p not in sys.path:
        sys.path.insert(0, _p)

import math
import numpy as np
import ml_dtypes
from contextlib import ExitStack

import concourse.bass as bass
import concourse.bacc as bacc
import concourse.mybir as mybir
from concourse.tile import TileContext, add_dep_helper
from concourse.bass_utils import run_bass_kernel_spmd

F32 = mybir.dt.float32
F16 = mybir.dt.float16
AF = mybir.ActivationFunctionType
ALU = mybir.AluOpType
PI4 = math.pi / 4.0

N_CORES = 8
B_FULL, D, C = 65536, 512, 10
B = B_FULL // N_CORES          # 8192 samples per core
N_QUBITS, Q_DEPTH = 4, 6

# x DMA slices along the sample axis (fine early for a fast pipeline start).
# Each (k, slice) gets its OWN SBUF tile: the tile framework tracks deps per
# tile, and a shared tile serializes compute reads against later DMA writes.
SLICES = [(0, 1024), (1024, 2560), (2560, 5120), (5120, 8192)]

# groups per phase-2 block (the xbar transposes only run after the ACT
# ring drains its k2/k3 plain copies, so the chains trail the stream; the
# two small final blocks shorten the last, fully-exposed chain)
BLOCK_GROUPS = [4, 4, 4, 2, 2]


def _slice_of(g):
    s0 = 512 * g
    for si, (c0, c1) in enumerate(SLICES):
        if c0 <= s0 < c1:
            return si, s0 - c0
    raise ValueError(g)


# ---------------------------------------------------------------- host math
def _apply_1q(state, gate, wire):
    state = np.moveaxis(state, wire, 0)
    state = np.tensordot(gate, state, axes=((1,), (0,)))
    return np.moveaxis(state, 0, wire)


def _apply_cnot(state, ctrl, tgt):
    state = np.moveaxis(state, (ctrl, tgt), (0, 1))
    state = np.stack([state[0], state[1][::-1]], axis=0)
    return np.moveaxis(state, (0, 1), (ctrl, tgt))


def _ry(theta):
    c, s = np.cos(theta * 0.5), np.sin(theta * 0.5)
    return np.array([[c, -s], [s, c]])


def _build_M(q_params: np.ndarray) -> np.ndarray:
    """16x16 matrix of the fixed part of the circuit (after the per-sample
    RY layer): 6 repetitions of [CNOT(0,1), CNOT(2,3), CNOT(1,2), RY layer]."""
    qw = np.asarray(q_params, np.float64).reshape(Q_DEPTH, N_QUBITS)
    M = np.zeros((16, 16), np.float64)
    for i in range(16):
        state = np.zeros(16, np.float64)
        state[i] = 1.0
        state = state.reshape((2,) * N_QUBITS)
        for k in range(Q_DEPTH):
            for a in range(0, N_QUBITS - 1, 2):
                state = _apply_cnot(state, a, a + 1)
            for a in range(1, N_QUBITS - 1, 2):
                state = _apply_cnot(state, a, a + 1)
            for w in range(N_QUBITS):
                state = _apply_1q(state, _ry(qw[k, w]), w)
        M[:, i] = state.reshape(16)
    return M


def _build_P(post_w: np.ndarray) -> np.ndarray:
    """P[i, c] = sum_w post_w[c, w] * z_w(i), where z_w(i) flips sign with
    bit (3-w) of the state index i (axis 0 of the state = qubit 0)."""
    post_w = np.asarray(post_w, np.float64)
    i = np.arange(16)
    z = np.stack([1.0 - 2.0 * ((i >> (3 - w)) & 1) for w in range(N_QUBITS)], 1)
    return z @ post_w.T  # [16, 10]


# ---------------------------------------------------------------- bass build
def build_nc() -> bass.Bass:
    # Bacc (not raw Bass): its finalize() runs generate_event_semaphores,
    # which splits multi-semaphore waits to satisfy the TRN2 one-wait-per-
    # instruction ISA limit.
    nc = bacc.Bacc(None)
    x4 = nc.dram_tensor("x4", [4, 128, B], F16, kind="ExternalInput")
    cst16 = nc.dram_tensor("cst16", [128, 352], F16, kind="ExternalInput")
    cst32 = nc.dram_tensor("cst32", [128, 132], F32, kind="ExternalInput")
    # transposed layout; host flips back (see unpack_out)
    out = nc.dram_tensor("out", [80, 1024], F32, kind="ExternalOutput")

    with ExitStack() as ctx:
        tc = ctx.enter_context(TileContext(nc))
        consts = ctx.enter_context(tc.tile_pool(name="consts", bufs=1))
        work = ctx.enter_context(tc.tile_pool(name="work", bufs=3))
        ps_po = ctx.enter_context(tc.tile_pool(name="ps_po", space="PSUM", bufs=3))
        ps2 = ctx.enter_context(tc.tile_pool(name="ps2", space="PSUM", bufs=4))

        # --- persistent SBUF ---
        cst16_sb = consts.tile([128, 352], F16)
        cst32_sb = consts.tile([128, 132], F32)
        xts = [[consts.tile([128, c1 - c0], F16, name=f"xt{k}_{si}")
                for si, (c0, c1) in enumerate(SLICES)] for k in range(4)]
        # rows 0:4 = qubits (rows 4:16 never written: engine writes must
        # start at partition 0; the xbar transpose just moves their bytes)
        t16 = consts.tile([16, B], F16)
        out2 = consts.tile([80, 128 * (B // 1024)], F32)
        warm = consts.tile([1, 2], F32)

        pre_wt = cst16_sb[:, 0:16]     # [p, 4k+q] = pre_w[q, 128k+p]
        mbd = cst16_sb[:, 16:144]      # block-diag 8 x M^T
        pbd = cst16_sb[:, 144:224]     # block-diag 8 x P
        pre_b = cst32_sb[0:4, 0:1]
        pb80 = cst32_sb[0:80, 1:2]
        bias_cos = cst32_sb[:, 2:3]    # 3*pi/4
        bias_sin = cst32_sb[:, 3:4]    # pi/4
        ident32 = cst32_sb[:, 4:132]

        # --- consts ride the gpsimd SWDGE path (own semaphore pool, off
        # the 8 shared HWDGE lanes)
        nc.gpsimd.dma_start(cst16_sb, cst16[:, :])
        nc.gpsimd.dma_start(cst32_sb, cst32[:, :])

        # --- the x stream: k0,k1 on the SP HWDGE ring; k2,k3 on the
        # gpsimd SWDGE queue (its own sem pool and ring).  The ACT HWDGE
        # ring carries ONLY the xbar transposes: a plain-copy -> transpose
        # transition drains the whole ring, so ANY plain copy there would
        # serialize every phase-2 chain behind the full x stream.
        for si, (c0, c1) in enumerate(SLICES):
            nc.scalar.dma_start(xts[2][si], x4[2, :, c0:c1])
            nc.scalar.dma_start(xts[3][si], x4[3, :, c0:c1])
            nc.sync.dma_start(xts[0][si], x4[0, :, c0:c1])
            nc.sync.dma_start(xts[1][si], x4[1, :, c0:c1])
            if si == 0:
                # pin the activation table to silu_and_others (the only
                # table with silu; it also has tanh+sin+square+identity, so
                # no further loads).  Reads `warm` itself -- garbage in,
                # garbage out, no DMA dep.
                nc.scalar.activation(warm[:, 0:1], warm[:, 1:2], AF.Silu)

        gbase = [sum(BLOCK_GROUPS[:b]) for b in range(len(BLOCK_GROUPS) + 1)]

        def phase1(blk):
            # pre-net for this block's groups of 512 samples
            for g in range(gbase[blk], gbase[blk + 1]):
                po = ps_po.tile([4, 512], F32, name="po", tag="po")
                si, o = _slice_of(g)
                for k in range(4):
                    nc.tensor.matmul(
                        po[:, :],
                        lhsT=pre_wt[:, 4 * k:4 * (k + 1)],
                        rhs=xts[k][si][:, o:o + 512],
                        start=(k == 0), stop=(k == 3))
                # PSUM evacuation + pre_b bias, fp16
                nc.vector.tensor_scalar(
                    out=t16[0:4, 512 * g:512 * (g + 1)],
                    in0=po[:, :], scalar1=pre_b, scalar2=None, op0=ALU.add)

        def phase2_front(blk):
            # trig + psi for this block; one xbar transpose:
            # [16, 512*ngroups] -> [128, (t, nt)(i, 16)]
            g0, g1 = gbase[blk], gbase[blk + 1]
            nt = 4 * (g1 - g0)              # sample tiles in this block
            th = work.tile([128, 16 * nt], F16, name="th", tag="th")
            nc.scalar.dma_start(
                th[:, :].rearrange("p (t i) -> p t i", i=16),
                t16[:, 512 * g0:512 * g1],
                transpose=True)
            th3 = th[:, :].rearrange("p (t i) -> p t i", i=16)[:, :, 0:4]
            tnh = work.tile([128, 4 * nt], F16, name="tnh", tag="tnh")
            nc.scalar.activation(
                tnh[:, :].rearrange("p (t q) -> p t q", q=4), th3, AF.Tanh)
            # cos(theta) = sin(pi/4*tanh + 3pi/4); sin(theta) = sin(.. + pi/4)
            cs = work.tile([128, 8 * nt], F32, name="cs", tag="cs")
            cs2 = cs[:, :].rearrange("p (t c e x) -> p t c e x", c=2, e=2, x=2)
            tnh3 = tnh[:, :].rearrange("p (t c e) -> p t c e", c=2, e=2)
            nc.scalar.activation(cs2[:, :, :, :, 0], tnh3, AF.Sin,
                                 bias=bias_cos, scale=PI4)
            nc.scalar.activation(cs2[:, :, :, :, 1], tnh3, AF.Sin,
                                 bias=bias_sin, scale=PI4)
            # v[t, c, a, b] = cs[t, qubit 2c, a] * cs[t, qubit 2c+1, b]
            # (c=0 -> qubits 01, c=1 -> qubits 23) in ONE broadcast multiply
            v = work.tile([128, 8 * nt], F32, name="v", tag="v")
            nc.vector.tensor_tensor(
                out=v[:, :].rearrange("p (t c a b) -> p t c a b", c=2, a=2, b=2),
                in0=cs2[:, :, :, 0, :].unsqueeze(4)
                    .broadcast_to((128, nt, 2, 2, 2)),
                in1=cs2[:, :, :, 1, :].unsqueeze(3)
                    .broadcast_to((128, nt, 2, 2, 2)),
                op=ALU.mult)
            v4 = v[:, :].rearrange("p (t c i) -> p t c i", c=2, i=4)
            psi = work.tile([128, 16 * nt], F32, name="psi", tag="psi")
            nc.vector.tensor_tensor(
                out=psi[:, :].rearrange("p (t a b) -> p t a b", a=4, b=4),
                in0=v4[:, :, 0, :].unsqueeze(3).broadcast_to((128, nt, 4, 4)),
                in1=v4[:, :, 1, :].unsqueeze(2).broadcast_to((128, nt, 4, 4)),
                op=ALU.mult)
            return psi

        def phase2_back(blk, psi):
            nh = (gbase[blk + 1] - gbase[blk]) // 2   # halves of 8 tiles
            for h in range(nh):
                hh = gbase[blk] // 2 + h
                psiT_ps = ps2.tile([128, 128], F32, name="psiT_ps", tag="p2")
                nc.tensor.transpose(
                    psiT_ps, psi[:, 128 * h:128 * (h + 1)], ident32)
                psiT = work.tile([128, 128], F16, name="psiT", tag="psiT")
                nc.vector.tensor_copy(psiT, psiT_ps)
                phi_ps = ps2.tile([128, 128], F32, name="phi_ps", tag="p2")
                nc.tensor.matmul(phi_ps, lhsT=mbd, rhs=psiT,
                                 start=True, stop=True)
                phi2 = work.tile([128, 128], F16, name="phi2", tag="phi2")
                nc.scalar.activation(phi2, phi_ps, AF.Square)
                o10 = ps2.tile([80, 128], F32, name="o10", tag="p2")
                nc.tensor.matmul(o10, lhsT=pbd, rhs=phi2,
                                 start=True, stop=True)
                nc.vector.tensor_scalar(
                    out=out2[:, 128 * hh:128 * (hh + 1)],
                    in0=o10[:, :], scalar1=pb80, scalar2=None, op0=ALU.add)

        def store(blk):
            h0, h1 = gbase[blk] // 2, gbase[blk + 1] // 2
            nc.sync.dma_start(out[:, 128 * h0:128 * h1],
                              out2[:, 128 * h0:128 * h1])

        NB = len(BLOCK_GROUPS)
        psis = {}
        phase1(0)
        psis[0] = phase2_front(0)
        for blk in range(1, NB):
            phase1(blk)
            phase2_back(blk - 1, psis.pop(blk - 1))
            store(blk - 1)
            psis[blk] = phase2_front(blk)
        phase2_back(NB - 1, psis.pop(NB - 1))
        store(NB - 1)

    nc.finalize()  # bacc: register alloc + event-semaphore wait splitting
    return nc


_NC_CACHE: dict = {}


def _get_nc() -> bass.Bass:
    if "nc" not in _NC_CACHE:
        _NC_CACHE["nc"] = build_nc()
    return _NC_CACHE["nc"]


def make_in_maps(inputs: dict) -> list:
    x = np.asarray(inputs["input_features"], np.float32)
    pre_w = np.asarray(inputs["pre_w"], np.float32)
    pre_b = np.asarray(inputs["pre_b"], np.float32)
    q_params = np.asarray(inputs["q_params"], np.float32)
    post_w = np.asarray(inputs["post_w"], np.float32)
    post_b = np.asarray(inputs["post_b"], np.float32)

    M = _build_M(q_params)
    P = _build_P(post_w)
    mbd = np.zeros((128, 128), np.float64)
    pbd = np.zeros((128, 80), np.float64)
    for t in range(8):
        mbd[16 * t:16 * (t + 1), 16 * t:16 * (t + 1)] = M.T
        pbd[16 * t:16 * (t + 1), 10 * t:10 * (t + 1)] = P
    # pre_wt[p, 4k+q] = pre_w[q, 128k+p]
    pre_wt = np.ascontiguousarray(
        pre_w.T.reshape(4, 128, 4).transpose(1, 0, 2).reshape(128, 16))
    cst16 = np.zeros((128, 352), np.float32)
    cst16[:, 0:16] = pre_wt
    cst16[:, 16:144] = mbd
    cst16[:, 144:224] = pbd
    cst16[:, 224:352] = np.eye(128)
    cst16 = cst16.astype(np.float16)

    cst32 = np.zeros((128, 132), np.float32)
    cst32[0:4, 0] = pre_b
    cst32[0:80, 1] = np.tile(post_b, 8)
    cst32[:, 2] = 3.0 * PI4
    cst32[:, 3] = PI4
    cst32[:, 4:132] = np.eye(128)
    cst32 = np.ascontiguousarray(cst32)

    xh = x.astype(np.float16)
    maps = []
    for i in range(N_CORES):
        xc = np.ascontiguousarray(xh[B * i:B * (i + 1)].T).reshape(4, 128, B)
        maps.append(dict(x4=xc, cst16=cst16, cst32=cst32))
    return maps


def unpack_out(dev: np.ndarray) -> np.ndarray:
    """[80, 1024] device layout -> [B, C].

    dev[10*t8 + c, 128*hh + s] is the output for class c of
    sample 128*(8*hh + t8) + s.
    """
    a = dev.reshape(8, 10, 8, 128)              # (t8, c, hh, s)
    return np.ascontiguousarray(
        a.transpose(2, 0, 3, 1).reshape(B, C))


def run_on_device(inputs: dict, **kwargs):
    """Returns (full_output, BassKernelResults)."""
    nc = _get_nc()
    in_maps = make_in_maps(inputs)
    res = run_bass_kernel_spmd(nc, in_maps, core_ids=list(range(N_CORES)),
                               **kwargs)
    full = np.concatenate(
        [unpack_out(res.results[i]["out"]) for i in range(N_CORES)], 0)
    return np.ascontiguousarray(full, dtype=np.float32), res


def kernel(**inputs) -> np.ndarray:
    out, _ = run_on_device(inputs)
    return out
